# revision 10
# baseline (speedup 1.0000x reference)
"""Trainium2 Bass kernel for nn_Attention2d (sparse_attention) — v6.

Reference (B=1): qkv = x @ Wqkv.T + bq; per (s,h): P = softmax_j(q.k^T,
mask); o = (P*bias) @ v; out = o @ Wout.T + bo.

Sharding: data-parallel over S (4 rows/core, 8 cores), no collectives.

Schedule (TimelineSim-driven; the ACT exp stream is the critical chain):
  * 32 (s,h) exp tiles/core; 28 exact on ACT (1145ns each), 4 as bf16
    Schraudolph (u16(A*logit+B) bit trick, ~3% rel err) split across
    DVE (jt0) + Pool (jt1-2) so the lg PSUM bank frees as fast as an ACT
    tile. A = 128/ln2 is folded into Wq host-side (ACT exps use
    scale=1/A). Softmax num/den share the error, so it mostly cancels;
    measured end-to-end err ~1.3e-2 (gate 2e-2).
  * x, W, identity ship in ONE merged DRAM region so the head needs few
    DMAs (HWDGE gen 625ns + 900ns sem each). Column order: x rows 0:512,
    all qk weights (q03|k03|q47|k47), x rows 512:1536, wv, wo.
  * Per-(s,h) slot: lg matmuls + exp + pbt + one small PE filler
    (qk-proj piece / paired v-proj / half epilogue chunk / out-proj),
    sized so PE's in-order queue never starves the lg->exp chain. PSUM
    pod rotation (bufs=2) is the scarce resource: v-projs pack 2 per
    bank, tail chunks borrow the then-dead lg pool banks.
  * Per-mt out-proj + per-mt output DMA; tail copies spread A/D/P.
"""

import ml_dtypes
import numpy as np

import concourse.bass as bass
import concourse.tile as tile
import concourse.mybir as mybir
from concourse import bacc
from concourse.bass_utils import run_bass_kernel_spmd

B, S, R, D = 1, 32, 384, 256
H, HD = 8, 32
NCORES = 8
SS = S // NCORES          # 4 sequence rows per core
M = SS * R                # 1536 rows per core
MT = M // 128             # 12
JT = R // 128             # 3
KT = D // 128             # 2
F32 = mybir.dt.float32
BF16 = mybir.dt.bfloat16
U16 = mybir.dt.uint16
AF = mybir.ActivationFunctionType
ALU = mybir.AluOpType

# Schraudolph constants (bf16 bit trick): pt_bits = u16(A*logit + B).
SCH_A = 128.0 / np.log(2.0)
SCH_B = 127.0 * 128.0 - 3.5

# ---- engine split knobs -------------------------------------------------
# NOTE: the GPSIMD (Pool) engine cannot access PSUM on hardware, so every
# PSUM reader (qk/v/oT/fo copies, odiv, exp/sch) must run on ACT or DVE.
# Pool only gets the SBUF-only pbt multiplies.
# exp engine per (s,h): "A"=ACT exact exp, "S"=Schraudolph on DVE
EXP_ENG = {}
for _s in range(SS):
    for _h in range(H):
        EXP_ENG[(_s, _h)] = "A"
for _sh in ((0, 3), (0, 6), (1, 2), (1, 6), (2, 3), (3, 1)):
    EXP_ENG[_sh] = "S"
# pbt engine per (s,h): "D"=DVE, "P"=Pool
PBT_ENG = {}
for _s in range(SS):
    for _h in range(H):
        PBT_ENG[(_s, _h)] = "P" if (_h % 2 == 0 and not (_s == 3 and _h == 6)) \
            else "D"
# qk copy engine per (nt, mc)
QK_COPY_ENG = {(0, 0): "A", (2, 0): "D", (1, 0): "D", (3, 0): "A",
               (0, 1): "A", (2, 1): "D", (1, 1): "A", (3, 1): "D",
               (0, 2): "D", (2, 2): "D", (1, 2): "D", (3, 2): "D"}
V_COPY_ENG = ["D"] * 12                 # per mt
ODIV_ENG = ["D"] * 12                   # per mt
OT_COPY_ENG = ["D"] * 9 + ["A", "D", "A"]
FO_COPY_ENG = ["A"] * 9 + ["A", "D", "A"]
N_WARM = 9                              # PE ramp warmup matmuls

# merged bf16 payload: per (partition, kt) rows of
#   [ x cols 0:512 | wqk 512 | x cols 512:1536 | wv 256 | wo 256 ]
XW_C = 2560
OB_XW = 0
OB_BIAS = OB_XW + 128 * 2 * XW_C           # [3,128,8,384] bias^T
OB_ID = OB_BIAS + JT * 128 * H * R         # [128,128] identity
NB = OB_ID + 128 * 128
# fp32 payload offsets
OF_BQ = 0                                  # [128,6]  b_qkv (nt-major)
OF_M01 = 128 * 6                           # [128,4,3] keep-mask
NF = OF_M01 + 128 * SS * JT
# generic-bias extras (separate small tensor, only when biases nonzero)
NG = 2 * D                                 # bv | bo as [2,256]

# column helpers inside xw
NTCOL = {0: 512, 2: 640, 1: 768, 3: 896}   # wqk bases (q03|k03|q47|k47)
WV_COL = 2048
WO_COL = 2304


def _xcol(m):
    """xw column for x row index m."""
    return m if m < 512 else 512 + m


def build_program(zero_bias: bool = True) -> bass.Bass:
    nc = bacc.Bacc("TRN2", target_bir_lowering=False, debug=False,
                   num_devices=NCORES)
    ab = nc.dram_tensor("allin_bf", [NB], BF16, kind="ExternalInput")
    af = nc.dram_tensor("allin_f32", [NF], F32, kind="ExternalInput")
    ag = None
    if not zero_bias:
        ag = nc.dram_tensor("allin_gb", [NG], F32, kind="ExternalInput")
    out_dram = nc.dram_tensor("out", [M, D], BF16, kind="ExternalOutput")
    with tile.TileContext(nc) as tc:
        _emit(nc, tc, ab, af, ag, out_dram, zero_bias)
    nc.compile()
    return nc


def _emit(nc, tc, ab, af, ag, out_dram, zero_bias):
    from contextlib import ExitStack
    ctx = ExitStack()
    with ctx:
        sg = ctx.enter_context(tc.tile_pool(name="sg", bufs=1))

        f32s = sg.tile([128, 6 + SS * JT], F32)
        xw = sg.tile([128, 2, XW_C], BF16)
        biasT = sg.tile([128, JT, H, R], BF16)
        ident = sg.tile([128, 128], BF16)

        xw_src = ab[OB_XW:OB_BIAS].rearrange("(p k c) -> p k c", p=128, k=2)
        bias_src = ab[OB_BIAS:OB_ID].rearrange(
            "(jt p h i) -> p jt h i", jt=JT, p=128, h=H)

        # ---- DMAs (sync engine; order = first-consumer order) ----
        nc.sync.dma_start(out=xw[:, :, 512:1024], in_=xw_src[:, :, 512:1024])
        nc.sync.dma_start(out=xw[:, :, 0:512], in_=xw_src[:, :, 0:512])
        nc.sync.dma_start(
            out=f32s[:], in_=af[:].rearrange("(p c) -> p c", p=128))
        nc.sync.dma_start(out=xw[:, :, 1024:2048],
                          in_=xw_src[:, :, 1024:2048])
        nc.sync.dma_start(out=biasT[:, :, 0:2, :], in_=bias_src[:, :, 0:2, :])
        nc.sync.dma_start(out=biasT[:, :, 2:4, :], in_=bias_src[:, :, 2:4, :])
        nc.sync.dma_start(out=biasT[:, :, 4:6, :], in_=bias_src[:, :, 4:6, :])
        nc.sync.dma_start(out=xw[:, :, 2048:2560],
                          in_=xw_src[:, :, 2048:2560])
        nc.sync.dma_start(out=biasT[:, :, 6:8, :], in_=bias_src[:, :, 6:8, :])
        nc.sync.dma_start(
            out=ident[:], in_=ab[OB_ID:NB].rearrange("(p c) -> p c", p=128))
        if not zero_bias:
            gbs = sg.tile([2, D], F32)
            nc.sync.dma_start(
                out=gbs[:], in_=ag[:].rearrange("(a b) -> a b", a=2))

        bq = f32s[:, 0:6]
        m01f = f32s[:, 6:6 + SS * JT].rearrange("p (s j) -> p s j", s=SS)
        m01b = sg.tile([128, SS, JT], BF16)
        nc.vector.tensor_copy(m01b[:], m01f)
        if not zero_bias:
            bv_bf = sg.tile([1, D], BF16)
            nc.vector.tensor_copy(bv_bf[:], gbs[0:1, :])
            bo_bf = sg.tile([1, D], BF16)
            nc.vector.tensor_copy(bo_bf[:], gbs[1:2, :])
            ones_k1 = sg.tile([1, 128], BF16)
            nc.vector.memset(ones_k1[:], 1.0)

        qkT = sg.tile([128, 4, M], BF16)
        vsb = sg.tile([128, MT, D], BF16)
        o_sb = sg.tile([128, MT, D], BF16)
        oT = sg.tile([128, KT, M], BF16)
        fo = sg.tile([128, MT, D], BF16)

        warm = sg.tile([128, 128], BF16)
        nc.vector.memset(warm[:], 0.125)
        podp = ctx.enter_context(tc.tile_pool(name="pod", bufs=2,
                                              space="PSUM"))
        lgp = ctx.enter_context(tc.tile_pool(name="lg", bufs=2,
                                             space="PSUM"))
        pt_pool = ctx.enter_context(tc.tile_pool(name="ptp", bufs=18))
        pbt_pool = ctx.enter_context(tc.tile_pool(name="pbtp", bufs=18))
        rec_pool = ctx.enter_context(tc.tile_pool(name="recp", bufs=4))

        def emit_warmup():
            for _ in range(N_WARM):
                pw = podp.tile([128, 512], F32, tag="pod")
                nc.tensor.matmul(pw[:, 0:128], warm[:], warm[:],
                                 start=True, stop=True)

        def emit_qk(nt, mc):
            pqk = podp.tile([128, 512], F32, tag="pod")
            col = NTCOL[nt]
            for kt in range(KT):
                nc.tensor.matmul(
                    pqk[:], xw[:, kt, col:col + 128],
                    xw[:, kt, _xcol(mc * 512):_xcol(mc * 512) + 512],
                    start=(kt == 0), stop=(kt == KT - 1))
            dst = qkT[:, nt, mc * 512:(mc + 1) * 512]
            eng = QK_COPY_ENG[(nt, mc)]
            if eng == "A":
                nc.scalar.copy(dst, pqk[:])
            elif zero_bias:
                if eng == "D":
                    nc.vector.tensor_copy(dst, pqk[:])
                else:
                    nc.gpsimd.tensor_copy(dst, pqk[:])
            else:
                if eng == "D":
                    nc.vector.tensor_scalar_add(dst, pqk[:],
                                                bq[:, nt:nt + 1])
                else:
                    nc.gpsimd.tensor_scalar(dst, pqk[:], bq[:, nt:nt + 1],
                                            None, ALU.add)

        def _v_mm_copy(pv, mt):
            s, jt = mt // JT, mt % JT
            if not zero_bias:
                nc.tensor.matmul(pv, ones_k1[:], bv_bf[:],
                                 start=True, stop=False)
            for kt in range(KT):
                nc.tensor.matmul(
                    pv, xw[:, kt, _xcol(mt * 128):_xcol(mt * 128) + 128],
                    xw[:, kt, WV_COL:WV_COL + D],
                    start=(zero_bias and kt == 0), stop=(kt == KT - 1))
            if V_COPY_ENG[mt] == "D":
                nc.vector.tensor_scalar_mul(vsb[:, mt, :], pv,
                                            m01f[:, s, jt:jt + 1])
            else:
                nc.gpsimd.tensor_scalar(vsb[:, mt, :], pv,
                                        m01f[:, s, jt:jt + 1], None,
                                        ALU.mult)

        def emit_v_pair(mt0):
            pvt = podp.tile([128, 512], F32, tag="pod")
            _v_mm_copy(pvt[:, 0:D], mt0)
            _v_mm_copy(pvt[:, D:2 * D], mt0 + 1)

        def emit_v_one(mt):
            pvt = podp.tile([128, 512], F32, tag="pod")
            _v_mm_copy(pvt[:, 0:D], mt)

        # per-s pt/pbt tile lists for the epilogues
        pts = {}
        pbts = {}

        def emit_tile(s, h):
            g, hp = h // 4, h % 4
            lg = lgp.tile([128, JT, 512], F32, tag="lg")
            for jt in range(JT):
                nc.tensor.matmul(
                    lg[:, jt, 0:R],
                    qkT[32 * hp:32 * hp + 32, 2 + g,
                        s * R + jt * 128:s * R + (jt + 1) * 128],
                    qkT[32 * hp:32 * hp + 32, g, s * R:(s + 1) * R],
                    start=True, stop=True,
                    tile_position=(32 * hp, 0))
            pt = pt_pool.tile([128, JT, R], BF16, tag="pt")
            pbt = pbt_pool.tile([128, JT, R], BF16, tag="pbt")
            if EXP_ENG[(s, h)] == "A":
                nc.scalar.activation(pt[:], lg[:, :, 0:R],
                                     AF.Exp, scale=float(1.0 / SCH_A))
            else:
                nc.vector.tensor_scalar(pt[:].bitcast(U16), lg[:, :, 0:R],
                                        float(SCH_B), None, ALU.add)
            if PBT_ENG[(s, h)] == "D":
                nc.vector.tensor_tensor(pbt[:], pt[:],
                                        biasT[:, :, h, :], ALU.mult)
            else:
                nc.gpsimd.tensor_tensor(pbt[:], pt[:],
                                        biasT[:, :, h, :], ALU.mult)
            pts[s].append(pt)
            pbts[s].append(pbt)

        chunk_pods = {}

        def _chunk_mms(pod, s, it, h0, h1):
            ib = slice(it * 128, (it + 1) * 128)
            for h in range(h0, h1):
                for jt in range(JT):
                    nc.tensor.matmul(
                        pod[:, 32 * h:32 * h + 32],
                        pbts[s][h][:, jt, ib],
                        vsb[:, 3 * s + jt, 32 * h:32 * h + 32],
                        start=(jt == 0), stop=(jt == JT - 1))
                for jt in range(JT):
                    nc.tensor.matmul(
                        pod[:, 256 + h:257 + h],
                        pts[s][h][:, jt, ib],
                        m01b[:, s, jt:jt + 1],
                        start=(jt == 0), stop=(jt == JT - 1))

        def epilogue_chunk_a(s, it, pool=None, tag="pod"):
            pod = (pool or podp).tile([128, 512] if tag == "pod"
                                      else [128, JT, 512], F32, tag=tag)
            if tag != "pod":
                pod = pod[:, 0, :]
            chunk_pods[(s, it)] = pod
            _chunk_mms(pod, s, it, 0, 4)

        def epilogue_chunk_b(s, it):
            pod = chunk_pods.pop((s, it))
            _chunk_mms(pod, s, it, 4, H)
            mt = 3 * s + it
            rec = rec_pool.tile([128, H], F32, tag="rec")
            nc.vector.reciprocal(rec[:], pod[:, 256:264])
            if ODIV_ENG[mt] == "D":
                nc.vector.tensor_tensor(
                    o_sb[:, mt, :].rearrange("p (a b) -> p a b", a=H),
                    pod[:, 0:256].rearrange("p (a b) -> p a b", a=H),
                    rec[:, :, None].broadcast_to([128, H, HD]), ALU.mult)
            else:
                nc.gpsimd.tensor_tensor(
                    o_sb[:, mt, :].rearrange("p (a b) -> p a b", a=H),
                    pod[:, 0:256].rearrange("p (a b) -> p a b", a=H),
                    rec[:, :, None].broadcast_to([128, H, HD]), ALU.mult)
            ptr = pod[:, 280:408].bitcast(BF16)  # [128,256] bf16
            for half in range(KT):
                nc.tensor.transpose(
                    ptr[:, half * 128:(half + 1) * 128],
                    o_sb[:, mt, half * 128:(half + 1) * 128],
                    ident[:])
            dst = oT[:, :, mt * 128:(mt + 1) * 128]
            src = ptr[:].rearrange("p (k c) -> p k c", k=KT)
            if OT_COPY_ENG[mt] == "D":
                nc.vector.tensor_copy(dst, src)
            elif OT_COPY_ENG[mt] == "A":
                nc.scalar.copy(dst, src)
            else:
                nc.gpsimd.tensor_copy(dst, src)

        def epilogue_out_mt(mt):
            s = mt // 3
            pf = podp.tile([128, 512], F32, tag="pod")
            if not zero_bias:
                nc.tensor.matmul(pf[:, 0:D], ones_k1[:], bo_bf[:],
                                 start=True, stop=False)
            for kt in range(KT):
                nc.tensor.matmul(
                    pf[:, 0:D], oT[:, kt, mt * 128:(mt + 1) * 128],
                    xw[:, kt, WO_COL:WO_COL + D],
                    start=(zero_bias and kt == 0),
                    stop=(kt == KT - 1))
            eng = FO_COPY_ENG[mt]
            if eng == "A":
                nc.scalar.copy(fo[:, mt, :], pf[:, 0:D])
            elif eng == "D":
                nc.vector.tensor_copy(fo[:, mt, :], pf[:, 0:D])
            else:
                nc.gpsimd.tensor_copy(fo[:, mt, :], pf[:, 0:D])
            a = mt % 3
            nc.sync.dma_start(
                out=out_dram[s * R + a * 128:s * R + (a + 1) * 128, :],
                in_=fo[:, mt, :])

        # ---- head: warmup + first projections ----
        emit_warmup()
        emit_qk(0, 0)
        emit_qk(2, 0)

        # PE filler work per (s,h) slot, emitted after the slot's tile.
        filler = {
            (0, 0): [lambda: emit_qk(1, 0)],
            (0, 1): [lambda: emit_qk(3, 0)],
            (0, 2): [lambda: emit_qk(0, 1)],
            (0, 3): [lambda: emit_qk(2, 1)],
            (0, 4): [lambda: emit_qk(1, 1)],
            (0, 5): [lambda: emit_qk(3, 1)],
            (0, 6): [lambda: emit_qk(0, 2), lambda: emit_v_pair(0)],
            (0, 7): [lambda: emit_qk(2, 2), lambda: emit_v_one(2)],
            (1, 0): [lambda: emit_qk(1, 2)],
            (1, 1): [lambda: emit_qk(3, 2)],
            (1, 2): [lambda: epilogue_chunk_a(0, 0)],
            (1, 3): [lambda: epilogue_chunk_b(0, 0)],
            (1, 4): [lambda: epilogue_chunk_a(0, 1), lambda: emit_v_pair(3)],
            (1, 5): [lambda: epilogue_chunk_b(0, 1), lambda: epilogue_out_mt(0)],
            (1, 6): [lambda: epilogue_chunk_a(0, 2), lambda: emit_v_one(5)],
            (1, 7): [lambda: epilogue_chunk_b(0, 2), lambda: epilogue_out_mt(1)],
            (2, 0): [lambda: epilogue_out_mt(2)],
            (2, 2): [lambda: epilogue_chunk_a(1, 0)],
            (2, 3): [lambda: epilogue_chunk_b(1, 0)],
            (2, 4): [lambda: epilogue_chunk_a(1, 1), lambda: emit_v_pair(6)],
            (2, 5): [lambda: epilogue_chunk_b(1, 1), lambda: epilogue_out_mt(3)],
            (2, 6): [lambda: epilogue_chunk_a(1, 2), lambda: emit_v_one(8)],
            (2, 7): [lambda: epilogue_chunk_b(1, 2), lambda: epilogue_out_mt(4)],
            (3, 0): [lambda: epilogue_out_mt(5)],
            (3, 2): [lambda: epilogue_chunk_a(2, 0)],
            (3, 3): [lambda: epilogue_chunk_b(2, 0), lambda: emit_v_pair(9)],
            (3, 4): [lambda: epilogue_chunk_a(2, 1), lambda: emit_v_one(11)],
            (3, 5): [lambda: epilogue_chunk_b(2, 1), lambda: epilogue_out_mt(6)],
            (3, 6): [lambda: epilogue_chunk_a(2, 2)],
            (3, 7): [lambda: epilogue_chunk_b(2, 2), lambda: epilogue_out_mt(7)],
        }

        for s in range(SS):
            pts[s] = []
            pbts[s] = []
            for h in range(H):
                emit_tile(s, h)
                for fn in filler.get((s, h), ()):
                    fn()

        # ---- tail: s=3 epilogue (chunk pods borrow the dead lg banks) ----
        epilogue_out_mt(8)
        for it in range(JT):
            epilogue_chunk_a(3, it, pool=lgp, tag="lg")
            epilogue_chunk_b(3, it)
        for mt in (9, 10, 11):
            epilogue_out_mt(mt)


def make_in_maps(pair_act, attention_mask, bias, W_qkv, b_qkv, W_out, b_out):
    bf = ml_dtypes.bfloat16
    pair = np.asarray(pair_act, np.float32)[0]          # [S,R,D]
    Wq = np.asarray(W_qkv, np.float32).copy()           # [768,256]
    Wq[0:256] *= SCH_A                                  # fold A into q
    Wo = np.asarray(W_out, np.float32)                  # [256,256]
    biasf = np.asarray(bias, np.float32)[0, 0]          # [H,R,R]
    mask01 = 1.0 - np.asarray(attention_mask, np.float32)[0]  # [S,R] keep

    # shared pieces; qk weight columns reordered q03|k03|q47|k47
    wq = Wq[0:256].T.reshape(2, 128, 256)               # (kt,p,nq)
    wk = Wq[256:512].T.reshape(2, 128, 256)
    wqk = np.concatenate([wq[:, :, 0:128], wk[:, :, 0:128],
                          wq[:, :, 128:256], wk[:, :, 128:256]], axis=2)
    wv = Wq[512:768].T.reshape(2, 128, D)               # (kt,p,dv)
    wo = Wo.T.reshape(2, 128, D)
    bias_t = np.ascontiguousarray(
        biasf.transpose(2, 0, 1)                        # [j,h,i]
        .reshape(JT, 128, H, R)).astype(bf)
    identity = np.eye(128, dtype=np.float32).astype(bf)
    bq6 = np.zeros((6, 128), np.float32)
    bq6.reshape(-1)[0:768] = np.asarray(b_qkv, np.float32)
    bq6 = bq6.T                                          # [128,6] nt-major

    gb = np.concatenate([np.asarray(b_qkv, np.float32)[512:768],
                         np.asarray(b_out, np.float32)])

    in_maps = []
    for c in range(NCORES):
        x = pair[c * SS:(c + 1) * SS].reshape(M, D)
        xT = x.T.reshape(2, 128, M)                     # (kt,p,m)
        xw = np.concatenate([xT[:, :, 0:512], wqk, xT[:, :, 512:M],
                             wv, wo], axis=2)           # (kt,p,2560)
        xw = np.ascontiguousarray(xw.transpose(1, 0, 2)).astype(bf)
        abf = np.concatenate([xw.ravel(), bias_t.ravel(), identity.ravel()])
        m01 = np.ascontiguousarray(
            mask01[c * SS:(c + 1) * SS].reshape(SS, JT, 128)
            .transpose(2, 0, 1))                         # [128,s,jt]
        afl = np.concatenate([bq6, m01.reshape(128, SS * JT)],
                             axis=1).ravel()             # [128,18] row-major
        assert abf.size == NB and afl.size == NF
        m = {"allin_bf": np.ascontiguousarray(abf.astype(bf)),
             "allin_f32": np.ascontiguousarray(afl.astype(np.float32))}
        zb = bool(np.all(np.asarray(b_qkv) == 0)
                  and np.all(np.asarray(b_out) == 0))
        if not zb:
            m["allin_gb"] = np.ascontiguousarray(gb)
        in_maps.append(m)
    return in_maps


_PROGRAM_CACHE = {}


def kernel(pair_act, attention_mask, bias, W_qkv, b_qkv, W_out, b_out,
           _want_results=False, **extra):
    in_maps = make_in_maps(pair_act, attention_mask, bias, W_qkv, b_qkv,
                           W_out, b_out)
    zero_bias = bool(np.all(np.asarray(b_qkv) == 0)
                     and np.all(np.asarray(b_out) == 0))
    key = ("nc", zero_bias)
    if key not in _PROGRAM_CACHE:
        _PROGRAM_CACHE[key] = build_program(zero_bias)
    nc = _PROGRAM_CACHE[key]
    res = run_bass_kernel_spmd(nc, in_maps, core_ids=list(range(NCORES)))
    out = np.concatenate(
        [np.asarray(r["out"], dtype=np.float32).reshape(SS, R, D)
         for r in res.results], axis=0)
    out = out.reshape(B, S, R, D)
    if _want_results:
        return out, res
    return out


# revision 36
# speedup vs baseline: 1.0697x; 1.0697x over previous
"""Trainium2 Bass kernel for nn_Attention2d (sparse_attention) — v3.

Reference (B=1): qkv = x @ Wqkv.T + bq; per (s,h): P = softmax_j(q.k^T,
mask); o = (P*bias) @ v; out = o @ Wout.T + bo.

Sharding: data-parallel over S (4 rows/core, 8 cores), no collectives.

Cost-model-driven design (TimelineSim is the scored metric):
  * matmul cost = out-free-size x pe_cycle. The o-matmul is FLIPPED:
    stationary = (P*bias)^T-block [j,128i], moving = v [j,32] -> out free 32
    (4x cheaper than the [d,384i] orientation). Softmax denominators are
    flipped likewise (stationary = P^T-block, moving = masked-ones column,
    out [128i,1] ~ free); o/den is fused into the PSUM->SBUF copy via a
    partition-broadcast reciprocal.
  * All host->device tensors are shipped PRE-TRANSPOSED / packed (xT, W
    pack, bias^T, identity) so no DMA transposes are needed; output is bf16
    (host upcasts). DMAs are few and ordered by first consumer; x/W are
    split so the first qk-proj tile lands ASAP.
  * exp is ACT-only (36.9k elems/partition is the critical chain); all
    PSUM->SBUF copies run on DVE except where ACT is provably idle (before
    the first exp and after the last one) - see *_ENG knobs. The bias
    multiply (pbt) splits DVE/Pool.
  * PE is kept continuously busy (warmup matmuls anchor the p-state ramp);
    per-s epilogues are deferred one s so exp never waits on them; the
    projection is emitted in consumer order (mc=0 slices first, nt 1/3
    during s=0's second head-group).
  * mask is applied by zeroing masked v rows at the v PSUM->SBUF copy and
    via the masked-ones den column; P itself is never masked (junk exps are
    harmless and get zero weight).
  * o transposed via PE-transpose (shipped identity) into a spare region of
    the same PSUM bank, then out-proj consumes oT.
"""

import ml_dtypes
import numpy as np

import concourse.bass as bass
import concourse.tile as tile
import concourse.mybir as mybir
from concourse import bacc
from concourse.bass_utils import run_bass_kernel_spmd

B, S, R, D = 1, 32, 384, 256
H, HD = 8, 32
NCORES = 8
SS = S // NCORES          # 4 sequence rows per core
M = SS * R                # 1536 rows per core
MT = M // 128             # 12
JT = R // 128             # 3
KT = D // 128             # 2
F32 = mybir.dt.float32
BF16 = mybir.dt.bfloat16
FP8 = mybir.dt.float8e4
U16 = mybir.dt.uint16
AF = mybir.ActivationFunctionType
ALU = mybir.AluOpType
PM = mybir.MatmulPerfMode

# Schraudolph constants (bf16 bit trick): pt_bits = u16(A*logit + BS)
SCH_A = 128.0 / np.log(2.0)          # folded into the q-copy scale
SCH_B = 127.0 * 128.0 - 3.5          # minimax offset (~+-3% rel err)

# ---- engine split knobs -------------------------------------------------
# exp engine per (s,h) tile: "A"=ACT exp, "D"=DVE schraudolph, "P"=Pool
EXP_ENG = ["A"] * 32
# pbt engine per (s,h): "D"=DVE, "P"=Pool
PBT_ENG = (["P", "P", "D", "D", "P", "D", "D", "D"]) * 4
QK_COPY_ENG = ["A"] + ["D"] * 5 + ["A"] + ["D"] * 5
V_COPY_ENG = ["D"] * 12               # 12 masked v copies
N_WARM = 9                            # PE ramp warmup matmuls
OT_COPY_ENG = ["D"] * 9 + ["A"] * 3   # late oT on idle ACT
FO_COPY_ENG = ["D"] * 6 + ["A", "A", "A"] + ["A", "D", "A"]
ODIV_ENG = ["D"] * 12                 # 12 po*rec copies
EPI_AT = 8   # head index where prev-s epilogue chunks are emitted (>=8: at s end)
STOP_AFTER = "full"  # dma|proj|exp|attn|trans|full (phase bisection probe)

# bf16 payload offsets
OB_XT = 0                                  # [128,2,M]
OB_WVWO = OB_XT + 128 * 2 * M              # [128,2,1024] (wq | wv | wo)
OB_BIAS = OB_WVWO + 128 * 2 * 1024         # [3,128,8,384] bias^T
OB_ID = OB_BIAS + JT * 128 * H * R         # [128,128] identity
NB = OB_ID + 128 * 128
# fp32 payload offsets
OF_BQ = 0                                  # [128,6]  b_qkv (nt-major)
OF_M01 = OF_BQ = 0
OF_M01 = 128 * 6                           # [128,4,3] keep-mask
NF = OF_M01 + 128 * SS * JT
# generic-bias extras (separate small tensor, only when biases nonzero)
NG = 2 * D                                 # bv | bo as [2,256]


def build_program(zero_bias: bool = True) -> bass.Bass:
    nc = bacc.Bacc("TRN2", target_bir_lowering=False, debug=False,
                   num_devices=NCORES)
    ab = nc.dram_tensor("allin_bf", [NB], BF16, kind="ExternalInput")
    af = nc.dram_tensor("allin_f32", [NF], F32, kind="ExternalInput")
    ag = None
    if not zero_bias:
        ag = nc.dram_tensor("allin_gb", [NG], F32, kind="ExternalInput")
    out_dram = nc.dram_tensor("out", [M, D], BF16, kind="ExternalOutput")
    with tile.TileContext(nc) as tc:
        _emit(nc, tc, ab, af, ag, out_dram, zero_bias)
    nc.compile()
    return nc


def _emit(nc, tc, ab, af, ag, out_dram, zero_bias):
    from contextlib import ExitStack
    ctx = ExitStack()
    with ctx:
        sg = ctx.enter_context(tc.tile_pool(name="sg", bufs=1))

        # ---- DMAs (sync engine; order = consumer order) ----
        f32s = sg.tile([128, 6 + SS * JT], F32)
        xT = sg.tile([128, 2, M], BF16)
        wvwo = sg.tile([128, 2, 1024], BF16)
        wv_src = ab[OB_WVWO:OB_BIAS].rearrange("(p k n) -> p k n",
                                               p=128, k=2)
        x_src = ab[OB_XT:OB_WVWO].rearrange("(p k m) -> p k m", p=128, k=2)
        nc.sync.dma_start(out=wvwo[:, :, 0:512], in_=wv_src[:, :, 0:512])
        nc.sync.dma_start(out=xT[:, :, 0:512], in_=x_src[:, :, 0:512])
        nc.sync.dma_start(
            out=f32s[:],
            in_=af[:].rearrange("(p c) -> p c", p=128))
        nc.sync.dma_start(out=xT[:, :, 512:1024], in_=x_src[:, :, 512:1024])
        nc.sync.dma_start(out=xT[:, :, 1024:M], in_=x_src[:, :, 1024:M])
        nc.sync.dma_start(out=wvwo[:, :, 512:1024],
                          in_=wv_src[:, :, 512:1024])
        biasT = sg.tile([128, JT, H, R], BF16)
        bias_src = ab[OB_BIAS:OB_ID].rearrange(
            "(jt p h i) -> p jt h i", jt=JT, p=128, h=H)
        nc.sync.dma_start(out=biasT[:, :, 0:4, :], in_=bias_src[:, :, 0:4, :])
        ident = sg.tile([128, 128], BF16)
        nc.sync.dma_start(
            out=ident[:],
            in_=ab[OB_ID:NB].rearrange("(p c) -> p c", p=128))
        nc.sync.dma_start(out=biasT[:, :, 4:8, :], in_=bias_src[:, :, 4:8, :])
        if not zero_bias:
            gbs = sg.tile([2, D], F32)
            nc.sync.dma_start(
                out=gbs[:], in_=ag[:].rearrange("(a b) -> a b", a=2))

        bq = f32s[:, 0:6]
        m01f = f32s[:, 6:6 + SS * JT].rearrange("p (s j) -> p s j", s=SS)
        m01b = sg.tile([128, SS, JT], BF16)
        nc.vector.tensor_copy(m01b[:], m01f)
        if not zero_bias:
            bv_bf = sg.tile([1, D], BF16)
            nc.vector.tensor_copy(bv_bf[:], gbs[0:1, :])
            bo_bf = sg.tile([1, D], BF16)
            nc.vector.tensor_copy(bo_bf[:], gbs[1:2, :])
            ones_k1 = sg.tile([1, 128], BF16)
            nc.vector.memset(ones_k1[:], 1.0)

        qkT = sg.tile([128, 4, M], BF16)
        vsb = sg.tile([128, MT, D], BF16)
        o_sb = sg.tile([128, MT, D], BF16)
        oT = sg.tile([128, KT, M], BF16)
        fo = sg.tile([128, MT, D], BF16)

        def _final_dma():
            nc.vector.memset(fo[:], 0.0)
            for s in range(SS):
                nc.sync.dma_start(
                    out=out_dram[s * R:(s + 1) * R, :].rearrange(
                        "(a p) d -> p a d", p=128),
                    in_=fo[:, 3 * s:3 * s + 3, :])
        if STOP_AFTER == "dma":
            _final_dma()
            return
        # ---- Phase 1 setup: proj emitters (called interleaved below) ----
        warm = sg.tile([128, 128], BF16)
        nc.vector.memset(warm[:], 0.125)
        podp = ctx.enter_context(tc.tile_pool(name="pod", bufs=2,
                                              space="PSUM"))
        lgp = ctx.enter_context(tc.tile_pool(name="lg", bufs=2,
                                             space="PSUM"))

        def emit_warmup():
            for i in range(N_WARM):
                pw = podp.tile([128, 512], F32, tag="pod")
                nc.tensor.matmul(pw[:, 0:128], warm[:], warm[:],
                                 start=True, stop=True)

        def emit_qk(pairs):
            for nt, mc in pairs:
                if True:
                    pqk = podp.tile([128, 512], F32, tag="pod")
                    for kt in range(KT):
                        nc.tensor.matmul(
                            pqk[:], wvwo[:, kt, nt * 128:(nt + 1) * 128],
                            xT[:, kt, mc * 512:(mc + 1) * 512],
                            start=(kt == 0), stop=(kt == KT - 1))
                    dst = qkT[:, nt, mc * 512:(mc + 1) * 512]
                    if QK_COPY_ENG[(nt * 3 + mc) % 12] == "A":
                        nc.scalar.copy(dst, pqk[:])
                    elif zero_bias:
                        nc.vector.tensor_copy(dst, pqk[:])
                    else:
                        nc.vector.tensor_scalar_add(dst, pqk[:],
                                                    bq[:, nt:nt + 1])

        def emit_v(mts):
            for mt in mts:
                s, jt = mt // JT, mt % JT
                pvt = podp.tile([128, 512], F32, tag="pod")
                pv = pvt[:, 0:D]
                if not zero_bias:
                    nc.tensor.matmul(pv, ones_k1[:], bv_bf[:],
                                     start=True, stop=False)
                for kt in range(KT):
                    nc.tensor.matmul(
                        pv, xT[:, kt, mt * 128:(mt + 1) * 128],
                        wvwo[:, kt, 512:512 + D],
                        start=(zero_bias and kt == 0), stop=(kt == KT - 1))
                if V_COPY_ENG[mt] == "A":
                    nc.scalar.activation(vsb[:, mt, :], pv, AF.Copy,
                                         scale=m01f[:, s, jt:jt + 1])
                else:
                    nc.vector.tensor_scalar_mul(vsb[:, mt, :], pv,
                                                m01f[:, s, jt:jt + 1])

        emit_warmup()
        emit_qk(((0, 0), (2, 0)))
        if STOP_AFTER == "proj":
            emit_qk([(nt, mc) for nt in (1, 3) for mc in range(3)]
                    + [(nt, mc) for nt in (0, 2) for mc in (1, 2)])
            emit_v(range(MT))
            _final_dma()
            return
        # ---- Phase 2: attention ----
        pt_pool = ctx.enter_context(tc.tile_pool(name="ptp", bufs=18))
        pbt_pool = ctx.enter_context(tc.tile_pool(name="pbtp", bufs=18))
        rec_pool = ctx.enter_context(tc.tile_pool(name="recp", bufs=4))
        def epilogue_chunk_p1(s, pts, pbts, it, pod):
            # o-matmuls for h0..h6, all dens, then the h7 o-matmul last so
            # rec only waits on pt(s,7) (exp), not pbt(s,7)
            ib = slice(it * 128, (it + 1) * 128)
            for h in range(H - 1):
                for jt in range(JT):
                    nc.tensor.matmul(
                        pod[:, 32 * h:32 * h + 32],
                        pbts[h][:, jt, ib],
                        vsb[:, 3 * s + jt, 32 * h:32 * h + 32],
                        start=(jt == 0), stop=(jt == JT - 1))
            for h in range(H):
                for jt in range(JT):
                    nc.tensor.matmul(
                        pod[:, 256 + h:257 + h],
                        pts[h][:, jt, ib],
                        m01b[:, s, jt:jt + 1],
                        start=(jt == 0), stop=(jt == JT - 1))
            for jt in range(JT):
                nc.tensor.matmul(
                    pod[:, 32 * 7:32 * 7 + 32],
                    pbts[7][:, jt, ib],
                    vsb[:, 3 * s + jt, 32 * 7:32 * 7 + 32],
                    start=(jt == 0), stop=(jt == JT - 1))
            rec = rec_pool.tile([128, H], F32, tag="rec")
            nc.vector.reciprocal(rec[:], pod[:, 256:264])
            return rec

        def epilogue_chunk_p2(s, it, pod, rec):
            mt = 3 * s + it
            nc.vector.tensor_tensor(
                o_sb[:, mt, :].rearrange("p (a b) -> p a b", a=H),
                pod[:, 0:256].rearrange("p (a b) -> p a b", a=H),
                rec[:, :, None].broadcast_to([128, H, HD]), ALU.mult)
            ptr = pod[:, 280:408].bitcast(BF16)  # [128,256] bf16
            for half in range(KT):
                nc.tensor.transpose(
                    ptr[:, half * 128:(half + 1) * 128],
                    o_sb[:, mt, half * 128:(half + 1) * 128],
                    ident[:])
            dst = oT[:, :, mt * 128:(mt + 1) * 128]
            if OT_COPY_ENG[mt] == "D":
                nc.vector.tensor_copy(
                    dst, ptr[:].rearrange("p (k c) -> p k c", k=KT))
            else:
                nc.scalar.copy(
                    dst, ptr[:].rearrange("p (k c) -> p k c", k=KT))

        def epilogue_chunk(s, pts, pbts, it):
            pod = podp.tile([128, 512], F32, tag="pod")
            rec = epilogue_chunk_p1(s, pts, pbts, it, pod)
            epilogue_chunk_p2(s, it, pod, rec)
        def epilogue_out(s, single_dma=False):
            if STOP_AFTER in ("attn", "trans"):
                return
            for mt in range(3 * s, 3 * s + 3):
                pf = podp.tile([128, 512], F32, tag="pod")
                if not zero_bias:
                    nc.tensor.matmul(pf[:, 0:D], ones_k1[:], bo_bf[:],
                                     start=True, stop=False)
                for kt in range(KT):
                    nc.tensor.matmul(
                        pf[:, 0:D], oT[:, kt, mt * 128:(mt + 1) * 128],
                        wvwo[:, kt, 768:768 + D],
                        start=(zero_bias and kt == 0),
                        stop=(kt == KT - 1))
                if FO_COPY_ENG[mt] == "A":
                    nc.scalar.copy(fo[:, mt, :], pf[:, 0:D])
                elif FO_COPY_ENG[mt] == "D":
                    nc.vector.tensor_copy(fo[:, mt, :], pf[:, 0:D])
                else:
                    nc.gpsimd.tensor_copy(fo[:, mt, :], pf[:, 0:D])
                if not single_dma:
                    a = mt % 3
                    nc.sync.dma_start(
                        out=out_dram[s * R + a * 128:s * R + (a + 1) * 128,
                                     :],
                        in_=fo[:, mt, :])
            if single_dma:
                nc.sync.dma_start(
                    out=out_dram[s * R:(s + 1) * R, :].rearrange(
                        "(a p) d -> p a d", p=128),
                    in_=fo[:, 3 * s:3 * s + 3, :])

        if True:
            prev = None
            for s in range(SS):
                pts = []
                pbts = []
                for h in range(H):
                    if s == 0 and h == 4:
                        emit_qk(((1, 0), (3, 0)))
                    if prev is not None and EPI_AT < H and h in (
                            EPI_AT, EPI_AT + 2, EPI_AT + 4):
                        epilogue_chunk(*prev, (h - EPI_AT) // 2)
                    g, hp = h // 4, h % 4
                    lg = lgp.tile([128, JT, 512], F32, tag="lg")
                    for jt in range(JT):
                        nc.tensor.matmul(
                            lg[:, jt, 0:R],
                            qkT[32 * hp:32 * hp + 32, 2 + g,
                                s * R + jt * 128:s * R + (jt + 1) * 128],
                            qkT[32 * hp:32 * hp + 32, g, s * R:(s + 1) * R],
                            start=True, stop=True,
                            tile_position=(32 * hp, 0))
                    pt = pt_pool.tile([128, JT, R], BF16, tag="pt")
                    ee = EXP_ENG[s * 8 + h]
                    if ee == "A":
                        nc.scalar.activation(pt[:], lg[:, :, 0:R], AF.Exp)
                    else:
                        eng = nc.vector if ee == "D" else nc.gpsimd
                        eng.tensor_scalar(pt[:].bitcast(U16), lg[:, :, 0:R],
                                          float(SCH_B), None, ALU.add)
                    pts.append(pt)
                    if STOP_AFTER == "expo":
                        continue
                    pbt = pbt_pool.tile([128, JT, R], BF16, tag="pbt")
                    if PBT_ENG[s * 8 + h] == "D":
                        nc.vector.tensor_tensor(pbt[:], pt[:],
                                                biasT[:, :, h, :], ALU.mult)
                    else:
                        nc.gpsimd.tensor_tensor(pbt[:], pt[:],
                                                biasT[:, :, h, :], ALU.mult)
                    pbts.append(pbt)
                if s == 0:
                    emit_qk([(nt, 1) for nt in (0, 2, 1, 3)])
                elif s == 1:
                    emit_qk([(nt, 2) for nt in (0, 2, 1, 3)])
                emit_v(range(3 * s, 3 * s + 3))
                if STOP_AFTER in ("exp", "expo"):
                    continue
                if prev is not None:
                    if EPI_AT >= H:
                        for it in range(JT):
                            epilogue_chunk(*prev, it)
                    epilogue_out(prev[0], single_dma=(prev[0] == SS - 2))
                prev = (s, pts, pbts)
            if prev is not None and STOP_AFTER not in ("exp", "expo"):
                # tail: all three chunks get independent PSUM (c0/c1 borrow
                # the now-dead lg banks) so their chains run concurrently;
                # matmul bulk first, then the per-chunk drains
                pods = []
                for it in range(JT):
                    if it < 2:
                        lgt = lgp.tile([128, JT, 512], F32, tag="lg")
                        pod = lgt[:, it, :]
                    else:
                        pod = podp.tile([128, 512], F32, tag="pod")
                    recv = epilogue_chunk_p1(prev[0], prev[1], prev[2], it,
                                             pod)
                    pods.append((pod, recv))
                for it in range(JT):
                    pod, recv = pods[it]
                    epilogue_chunk_p2(prev[0], it, pod, recv)
                epilogue_out(prev[0], single_dma=True)
        if STOP_AFTER in ("expo", "exp", "attn", "trans"):
            _final_dma()


def make_in_maps(pair_act, attention_mask, bias, W_qkv, b_qkv, W_out, b_out):
    bf = ml_dtypes.bfloat16
    pair = np.asarray(pair_act, np.float32)[0]          # [S,R,D]
    Wq = np.asarray(W_qkv, np.float32)                  # [768,256]
    Wo = np.asarray(W_out, np.float32)                  # [256,256]
    biasf = np.asarray(bias, np.float32)[0, 0]          # [H,R,R]
    mask01 = 1.0 - np.asarray(attention_mask, np.float32)[0]  # [S,R] keep

    # shared pieces
    wqk = Wq[0:512].T.reshape(2, 128, 512)              # (kt,p,n)
    wv = Wq[512:768].T.reshape(2, 128, D)               # (kt,p,dv)
    wo = Wo.T.reshape(2, 128, D)
    wvwo = np.ascontiguousarray(
        np.concatenate([wqk, wv, wo], axis=2).transpose(1, 0, 2)).astype(bf)
    bias_t = np.ascontiguousarray(
        biasf.transpose(2, 0, 1)                        # [j,h,i]
        .reshape(JT, 128, H, R)).astype(bf)
    identity = np.eye(128, dtype=np.float32).astype(bf)
    shared_bf = np.concatenate(
        [wvwo.ravel(), bias_t.ravel(), identity.ravel()])
    bq6 = np.zeros((6, 128), np.float32)
    bq6.reshape(-1)[0:768] = np.asarray(b_qkv, np.float32)
    bq6 = bq6.T                                          # [128,6] nt-major

    gb = np.concatenate([np.asarray(b_qkv, np.float32)[512:768],
                         np.asarray(b_out, np.float32)])

    in_maps = []
    for c in range(NCORES):
        x = pair[c * SS:(c + 1) * SS].reshape(M, D)
        xT = np.ascontiguousarray(x.T.reshape(2, 128, M).transpose(1, 0, 2))
        abf = np.concatenate([xT.astype(bf).ravel(), shared_bf])
        m01 = np.ascontiguousarray(
            mask01[c * SS:(c + 1) * SS].reshape(SS, JT, 128)
            .transpose(2, 0, 1))                         # [128,s,jt]
        afl = np.concatenate([bq6, m01.reshape(128, SS * JT)],
                             axis=1).ravel()             # [128,18] row-major
        assert abf.size == NB and afl.size == NF
        m = {"allin_bf": np.ascontiguousarray(abf.astype(bf)),
             "allin_f32": np.ascontiguousarray(afl.astype(np.float32))}
        zb = bool(np.all(np.asarray(b_qkv) == 0)
                  and np.all(np.asarray(b_out) == 0))
        if not zb:
            m["allin_gb"] = np.ascontiguousarray(gb)
        in_maps.append(m)
    return in_maps


_PROGRAM_CACHE = {}


def kernel(pair_act, attention_mask, bias, W_qkv, b_qkv, W_out, b_out,
           _want_results=False, **extra):
    in_maps = make_in_maps(pair_act, attention_mask, bias, W_qkv, b_qkv,
                           W_out, b_out)
    zero_bias = bool(np.all(np.asarray(b_qkv) == 0)
                     and np.all(np.asarray(b_out) == 0))
    key = ("nc", zero_bias)
    if key not in _PROGRAM_CACHE:
        _PROGRAM_CACHE[key] = build_program(zero_bias)
    nc = _PROGRAM_CACHE[key]
    res = run_bass_kernel_spmd(nc, in_maps, core_ids=list(range(NCORES)))
    out = np.concatenate(
        [np.asarray(r["out"], dtype=np.float32).reshape(SS, R, D)
         for r in res.results], axis=0)
    out = out.reshape(B, S, R, D)
    if _want_results:
        return out, res
    return out
